# revision 14
# baseline (speedup 1.0000x reference)
"""Trainium2 Bass kernel for nn_CLSTM: 2-layer complex LSTM.

T=256 time steps, B=64 batch, H=512 complex hidden (1024 real dims), L=2 layers.

Strategy (8 NeuronCores, tensor-parallel over hidden channels):
  - Each core owns 64 complex channels (=128 real dims) of every gate / state.
  - Activations live batch-on-partitions: tiles are [64 batch, channels-free],
    so all complex (r,i) arithmetic is free-dim local (DVE lanes are
    partition-local on trn2 - no cross-partition ops needed).
  - Gate GEMMs: out[64, 512] (batch x 4*128 gate cols) accumulated over
    K-tiles; lhsT = transposed activations [128 chan, 64 batch] (stationary),
    rhs = weight slices [128, 512] (moving, N=512 streams at 1 col/cycle).
  - Complex linear algebra is lifted to real GEMMs with the doubled matrix
    W_big = [[wr, wi], [-wi, wr]].
  - Bias enters as a K=1 matmul of a ones-row against a bias-row.
  - h is transposed back to [128 chan, 64 batch] on the PE each step, cast,
    and AllGather'ed across the 8 cores (ncfw collective via DRAM bounce).
  - Output projection is batched 8 steps at a time (M=128 = 2 steps x 64).
  - All input/output permutation, transposition and weight packing is done
    host-side in numpy.
"""

import os
import sys
from contextlib import ExitStack

sys.path.insert(0, "/opt/trn_rl_repo")

import numpy as np
import ml_dtypes

import concourse.bass as bass
import concourse.tile as tile
import concourse.mybir as mybir
from concourse import bacc

# Problem constants
T_FULL, B, L, H = 256, 64, 2, 512
D = 2 * H            # 1024 real dims
NCORES = 8
CPC = H // NCORES    # 64 complex channels per core
RPC = 2 * CPC        # 128 real dims per core
KT_H = D // 128      # 8 K-tiles for one hidden vector
GATE_ORDER = (0, 1, 3, 2)   # col-block order: f, i, o, a (ref order 0=f,1=i,2=a,3=o)
OUT_CHUNK = 8        # outproj batching (steps per chunk)

F32 = mybir.dt.float32
BF16 = mybir.dt.bfloat16

WDT_NAME = os.environ.get("CLSTM_WDT", "f32r")   # "f32" | "f32r" | "bf16"
COMM_MODE = os.environ.get("CLSTM_COMM", "merged")  # merged | ccompute | none


def _perm():
    """sharded row order -> standard feature index."""
    p = np.zeros(D, dtype=np.int64)
    for k in range(NCORES):
        p[128 * k:128 * k + 64] = np.arange(64 * k, 64 * k + 64)
        p[128 * k + 64:128 * k + 128] = 512 + np.arange(64 * k, 64 * k + 64)
    return p


PERM = _perm()


def _wbig(w):
    """w: [2, H, H] (wr, wi) -> [2H, 2H] real matrix, std order both sides."""
    wr, wi = np.asarray(w[0]), np.asarray(w[1])
    return np.block([[wr, wi], [-wi, wr]]).astype(np.float32)


def _np_wdt(name):
    return {"f32": np.float32, "f32r": np.float32, "bf16": ml_dtypes.bfloat16}[name]


def pack_inputs(x, h0, c0, Uw, Ub, Ww, Wb, Wout, bout, T, wdt_name):
    """Build the 8 per-core input maps."""
    x = np.asarray(x, np.float32)[:T]
    h0 = np.asarray(h0, np.float32)
    c0 = np.asarray(c0, np.float32)
    Uw = np.asarray(Uw, np.float32)
    Ub = np.asarray(Ub, np.float32)
    Ww = np.asarray(Ww, np.float32)
    Wb = np.asarray(Wb, np.float32)
    Wout = np.asarray(Wout, np.float32)
    bout = np.asarray(bout, np.float32)
    nwdt = _np_wdt(wdt_name)

    # xts[t, p, kk*64+b] = x[t, b, PERM[kk*128+p]]  (shared by all cores)
    xp = x[:, :, PERM]                                    # [T, B, D]
    xts = np.ascontiguousarray(
        xp.reshape(T, B, KT_H, 128).transpose(0, 3, 2, 1).reshape(T, 128, KT_H * B)
    ).astype(nwdt)

    # hg[l, p, kk*64+b] = h0[l, b, PERM[kk*128+p]]
    h0p = h0[:, :, PERM]
    hg = np.ascontiguousarray(
        h0p.reshape(L, B, KT_H, 128).transpose(0, 3, 2, 1).reshape(L, 128, KT_H * B)
    ).astype(nwdt)

    ones = np.ones((1, 128), dtype=nwdt)
    ident = np.eye(64, dtype=np.float32)

    # Big matrices (std order), per layer/gate
    Ubig = [[_wbig(Uw[l, g]) for g in range(4)] for l in range(L)]
    Wbig = [[_wbig(Ww[l, g]) for g in range(4)] for l in range(L)]
    Obig = _wbig(Wout)

    in_maps = []
    for k in range(NCORES):
        colsel = PERM[128 * k:128 * k + 128]
        ws = []
        for l in range(L):
            cols = []
            for g in GATE_ORDER:
                top = Ubig[l][g][PERM][:, colsel]      # [1024, 128] input-x part
                bot = Wbig[l][g][PERM][:, colsel]      # [1024, 128] recurrent part
                cols.append(np.concatenate([top, bot], axis=0))   # [2048, 128]
            wcat = np.concatenate(cols, axis=1)        # [2048, 512]
            w = wcat.reshape(16, 128, 512).transpose(1, 0, 2).reshape(128, 16 * 512)
            ws.append(np.ascontiguousarray(w).astype(nwdt))

        brow = np.zeros((1, 2 * 512 + 128), dtype=np.float32)
        for l in range(L):
            bsum = Ub[l] + Wb[l]                       # [4, D]
            brow[0, 512 * l:512 * (l + 1)] = np.concatenate(
                [bsum[g][colsel] for g in GATE_ORDER])
        brow[0, 1024:1152] = bout[colsel]

        wo = Obig[PERM][:, colsel]                     # [1024, 128]
        wo = wo.reshape(8, 128, 128).transpose(1, 0, 2).reshape(128, 1024)

        in_maps.append({
            "xts": xts,
            "w0": ws[0],
            "w1": ws[1],
            "wo": np.ascontiguousarray(wo).astype(nwdt),
            "brs": brow.astype(nwdt),
            "hg": hg,
            "c0s": np.ascontiguousarray(c0[:, :, colsel]),
            "ones": ones,
            "ident": ident,
        })
    return in_maps


def decode_outputs(results, T):
    """results: list of 8 per-core out dicts -> (y, hn, cn) full arrays."""
    y = np.zeros((T, B, D), dtype=np.float32)
    hn = np.zeros((L, B, D), dtype=np.float32)
    cn = np.zeros((L, B, D), dtype=np.float32)
    nch = T // OUT_CHUNK
    for k in range(NCORES):
        colsel = PERM[128 * k:128 * k + 128]
        yt = results[k]["yt"]                          # [nch, 128, 512]
        # yt[c, s2*64+b, 128*j+m] = y[8c+2j+s2, b, colsel[m]]
        yy = yt.reshape(nch, 2, 64, 4, 128).transpose(0, 3, 1, 2, 4)
        y[:, :, colsel] = yy.reshape(T, 64, 128)
        hn[:, :, colsel] = results[k]["hns"]
        cn[:, :, colsel] = results[k]["cns"]
    return y, hn, cn


def build_nc(T, wdt_name):
    f32r_mode = wdt_name == "f32r"
    wdt = {"bf16": BF16, "f32": F32, "f32r": mybir.dt.float32r}[wdt_name]

    def mc(ap):      # matmul-dtype cast (no-op; dtypes carried on tensors)
        return ap

    def swapview(ap):
        """[P, 2h] unit-stride AP -> view reading (2nd half | 1st half)."""
        (pstep, pcnt), (fstep, fcnt) = list(ap.ap)
        assert fstep == 1 and fcnt % 2 == 0
        h = fcnt // 2
        return bass.AP(ap.tensor, ap.offset + h,
                       [[pstep, pcnt], [-h, 2], [1, h]])

    nch = T // OUT_CHUNK
    nc = bacc.Bacc("TRN2", target_bir_lowering=False, debug=False,
                   num_devices=NCORES)

    xts = nc.dram_tensor("xts", [T, 128, 512], wdt, kind="ExternalInput").ap()
    w0 = nc.dram_tensor("w0", [128, 16 * 512], wdt, kind="ExternalInput").ap()
    w1 = nc.dram_tensor("w1", [128, 16 * 512], wdt, kind="ExternalInput").ap()
    wo = nc.dram_tensor("wo", [128, 8 * 128], wdt, kind="ExternalInput").ap()
    brs = nc.dram_tensor("brs", [1, 1152], wdt, kind="ExternalInput").ap()
    hg = nc.dram_tensor("hg", [L, 128, 512], wdt, kind="ExternalInput").ap()
    c0s = nc.dram_tensor("c0s", [L, 64, 128], F32, kind="ExternalInput").ap()
    onesd = nc.dram_tensor("ones", [1, 128], wdt, kind="ExternalInput").ap()
    identd = nc.dram_tensor("ident", [64, 64], F32, kind="ExternalInput").ap()

    yt = nc.dram_tensor("yt", [nch, 128, 512], F32, kind="ExternalOutput").ap()
    hns = nc.dram_tensor("hns", [L, 64, 128], F32, kind="ExternalOutput").ap()
    cns = nc.dram_tensor("cns", [L, 64, 128], F32, kind="ExternalOutput").ap()

    rg = [list(range(NCORES))]

    with tile.TileContext(nc) as tc, ExitStack() as ctx:
        consts = ctx.enter_context(tc.tile_pool(name="consts", bufs=1))
        w0s = consts.tile([128, 16 * 512], wdt)
        nc.sync.dma_start(w0s[:], w0)
        w1s = consts.tile([128, 16 * 512], wdt)
        nc.sync.dma_start(w1s[:], w1)
        wos = consts.tile([128, 8 * 128], wdt)
        nc.sync.dma_start(wos[:], wo)
        brss = consts.tile([1, 1152], wdt)
        nc.sync.dma_start(brss[:], brs)
        ones = consts.tile([1, 128], wdt)
        nc.sync.dma_start(ones[:], onesd)
        ident = consts.tile([64, 64], F32)
        nc.sync.dma_start(ident[:], identd)
        hg0s = consts.tile([128, 512], wdt)
        nc.sync.dma_start(hg0s[:], hg[0])
        hg1s = consts.tile([128, 512], wdt)
        nc.sync.dma_start(hg1s[:], hg[1])

        xpool = ctx.enter_context(tc.tile_pool(name="xst", bufs=4))
        psc = ctx.enter_context(tc.tile_pool(name="psc", bufs=2, space="PSUM"))
        pst_p = ctx.enter_context(tc.tile_pool(name="pst", bufs=2, space="PSUM"))
        psy = ctx.enter_context(tc.tile_pool(name="psy", bufs=1, space="PSUM"))
        gpool = ctx.enter_context(tc.tile_pool(name="gates", bufs=2))
        tmp = ctx.enter_context(tc.tile_pool(name="tmp", bufs=2))
        cpool = ctx.enter_context(tc.tile_pool(name="cstate", bufs=2))
        hpool = ctx.enter_context(tc.tile_pool(name="hstate", bufs=2))
        htp = ctx.enter_context(tc.tile_pool(name="ht", bufs=2))
        g0pool = ctx.enter_context(tc.tile_pool(name="g0", bufs=2))
        chpool = ctx.enter_context(tc.tile_pool(name="chunk", bufs=2))
        ypool = ctx.enter_context(tc.tile_pool(name="ysb", bufs=2))
        dbin = ctx.enter_context(tc.tile_pool(name="dbin", bufs=3, space="DRAM"))
        dbout = ctx.enter_context(tc.tile_pool(name="dbout", bufs=3, space="DRAM"))

        # initial c states
        c_prev = []
        for l in range(L):
            ct = cpool.tile([64, 128], F32)
            nc.sync.dma_start(ct[:], c0s[l])
            c_prev.append(ct)

        Sig = mybir.ActivationFunctionType.Sigmoid
        Tanh = mybir.ActivationFunctionType.Tanh

        def cell(lhsT_tiles, wsb, boff, c_old):
            """One complex-LSTM cell for this core's channel slice.

            lhsT_tiles: list of 16 APs [128, 64] (transposed inputs, wdt)
            wsb: weight SBUF tile [128, 16*512]; boff: bias col offset
            c_old: [64, 128] f32
            returns (c_new, h [64,128] f32, hT [128,64] wdt)
            """
            ps = psc.tile([64, 512], F32)
            nc.tensor.matmul(ps[:], mc(ones[0:1, 0:64]), mc(brss[0:1, boff:boff + 512]),
                             start=True, stop=False)
            for kt in range(16):
                nc.tensor.matmul(ps[:], mc(lhsT_tiles[kt]),
                                 mc(wsb[:, 512 * kt:512 * (kt + 1)]),
                                 start=False, stop=(kt == 15))
            gt = gpool.tile([64, 512], F32)
            nc.scalar.activation(gt[:, 0:384], ps[:, 0:384], Sig)
            nc.scalar.activation(gt[:, 384:512], ps[:, 384:512], Tanh)
            f = gt[:, 0:128]
            ig = gt[:, 128:256]
            o = gt[:, 256:384]
            a = gt[:, 384:512]

            c, v = c_old, nc.vector
            A = tmp.tile([64, 128], F32)
            v.tensor_mul(A[:], c[:], f)                       # (crfr | cifi)
            Q = tmp.tile([64, 128], F32)
            v.tensor_mul(Q[:], a, ig)                         # (arir | aiii)
            V1 = tmp.tile([64, 128], F32)
            v.tensor_add(V1[:], A[:], Q[:])
            CX = tmp.tile([64, 128], F32)
            v.tensor_mul(CX[:], swapview(c[:]), f)            # (cifr | crfi)
            DX = tmp.tile([64, 128], F32)
            v.tensor_mul(DX[:], swapview(a), ig)              # (aiir | arii)
            V2 = tmp.tile([64, 128], F32)
            v.tensor_add(V2[:], CX[:], DX[:])
            c_new = cpool.tile([64, 128], F32)
            v.tensor_sub(c_new[:, 0:64], V1[:, 0:64], V1[:, 64:128])
            v.tensor_add(c_new[:, 64:128], V2[:, 0:64], V2[:, 64:128])

            th = tmp.tile([64, 128], F32)
            nc.scalar.activation(th[:], c_new[:], Tanh)
            E = tmp.tile([64, 128], F32)
            v.tensor_mul(E[:], o, th[:])                      # (or*thr | oi*thi)
            FX = tmp.tile([64, 128], F32)
            v.tensor_mul(FX[:], swapview(o), th[:])           # (oithr | orthi)
            h = hpool.tile([64, 128], F32)
            v.tensor_sub(h[:, 0:64], E[:, 0:64], E[:, 64:128])
            v.tensor_add(h[:, 64:128], FX[:, 0:64], FX[:, 64:128])

            pt = pst_p.tile([128, 64], F32)
            nc.tensor.transpose(pt[:], h[:], ident[:])
            hT = htp.tile([128, 64], wdt)
            nc.scalar.copy(hT[:], pt[:])
            return c_new, h, hT

        def allgather(hT, dest_ap):
            """hT [128,64] sbuf -> gathered into dest_ap ([128, 8, 64] view)."""
            if COMM_MODE == "none":
                # timing-only stub: local broadcast (WRONG results)
                for kk in range(NCORES):
                    nc.sync.dma_start(dest_ap[:, kk, :], hT[:])
                return
            bi = dbin.tile([128, 64], wdt)
            nc.sync.dma_start(bi[:], hT[:])
            bo = dbout.tile([NCORES * 128, 64], wdt)
            nc.gpsimd.collective_compute(
                "AllGather", mybir.AluOpType.bypass, replica_groups=rg,
                ins=[bi[:].opt()], outs=[bo[:].opt()])
            nc.sync.dma_start(
                dest_ap, bo[:].rearrange("(kk p) b -> p kk b", kk=NCORES))

        if COMM_MODE == "merged":
            # Skewed schedule: one AllGather per iteration carrying
            # (h_l0(i), h_l1(i-1)).
            g0_ring = {}        # i -> gathered h_l0(i) [128, 512] tile
            chunks = []
            h_last = [None, None]
            c_last = [None, None]
            hT0_i = hT1_prev = None
            for i in range(T + 1):
                if i >= 1 and (i - 1) % 8 == 0:
                    chunks.append(chpool.tile([128, KT_H * OUT_CHUNK * 64],
                                              wdt, name="chunk", tag="chunk"))
                if i < T:
                    xt = xpool.tile([128, 512], wdt)
                    nc.sync.dma_start(xt[:], xts[i])
                    g0src = hg0s if i == 0 else g0_ring[i - 1]
                    lhsT0 = [xt[:, 64 * kk:64 * (kk + 1)] for kk in range(KT_H)] + \
                            [g0src[:, 64 * kk:64 * (kk + 1)] for kk in range(KT_H)]
                    c0n, h0t, hT0_i = cell(lhsT0, w0s, 0, c_prev[0])
                    c_prev[0] = c0n
                    h_last[0], c_last[0] = h0t, c0n
                if i >= 1:
                    g0 = g0_ring[i - 1]
                    if i == 1:
                        l1rec = [hg1s[:, 64 * kk:64 * (kk + 1)]
                                 for kk in range(KT_H)]
                    else:
                        pch = chunks[(i - 2) // OUT_CHUNK]
                        ps_ = (i - 2) % OUT_CHUNK
                        l1rec = [pch[:, 512 * kk + 64 * ps_:512 * kk + 64 * ps_ + 64]
                                 for kk in range(KT_H)]
                    lhsT1 = [g0[:, 64 * kk:64 * (kk + 1)]
                             for kk in range(KT_H)] + l1rec
                    c1n, h1t, hT1_prev = cell(lhsT1, w1s, 512, c_prev[1])
                    c_prev[1] = c1n
                    h_last[1], c_last[1] = h1t, c1n

                # merged AllGather: slot 0 = h_l0(i), slot 1 = h_l1(i-1)
                bi = dbin.tile([256, 64], wdt, name="bi", tag="bi")
                nc.sync.dma_start(bi[0:128, :],
                                  (hT0_i if i < T else hT1_prev)[:])
                nc.sync.dma_start(bi[128:256, :],
                                  (hT1_prev if i >= 1 else hT0_i)[:])
                bo = dbout.tile([NCORES * 256, 64], wdt, name="bo", tag="bo")
                nc.gpsimd.collective_compute(
                    "AllGather", mybir.AluOpType.bypass, replica_groups=rg,
                    ins=[bi[:].opt()], outs=[bo[:].opt()])
                bview = bo[:].rearrange("(kk s p) b -> p s kk b",
                                        kk=NCORES, s=2)
                if i < T:
                    g0 = g0pool.tile([128, 512], wdt)
                    nc.sync.dma_start(
                        g0[:].rearrange("p (kk b) -> p kk b", kk=KT_H),
                        bview[:, 0])
                    g0_ring[i] = g0
                    g0_ring.pop(i - 3, None)
                if i >= 1:
                    j = i - 1
                    ch = chunks[j // OUT_CHUNK]
                    nc.sync.dma_start(
                        ch[:].rearrange("p (kk s b) -> p kk s b",
                                        kk=KT_H, s=OUT_CHUNK)[:, :, j % 8, :],
                        bview[:, 1])
                # outproj for chunk c once its last slot (step 8c+7) landed
                if i >= 8 and i % 8 == 0:
                    ci = (i - 8) // OUT_CHUNK
                    chunk = chunks[ci]
                    py = psy.tile([128, 512], F32)
                    for j in range(4):
                        nc.tensor.matmul(py[:, 128 * j:128 * (j + 1)],
                                         mc(ones[0:1, 0:128]),
                                         mc(brss[0:1, 1024:1152]),
                                         start=True, stop=False)
                        for kt in range(KT_H):
                            lh = chunk[:, 512 * kt + 128 * j:
                                       512 * kt + 128 * (j + 1)]
                            nc.tensor.matmul(py[:, 128 * j:128 * (j + 1)],
                                             mc(lh),
                                             mc(wos[:, 128 * kt:128 * (kt + 1)]),
                                             start=False, stop=(kt == KT_H - 1))
                    ysb = ypool.tile([128, 512], F32)
                    nc.scalar.activation(ysb[:], py[:], Tanh)
                    nc.sync.dma_start(yt[ci], ysb[:])

            for l in range(L):
                nc.sync.dma_start(hns[l], h_last[l][:])
                nc.sync.dma_start(cns[l], c_last[l][:])

        g0_prev = None          # gathered h for layer 0, [128, 512] tile
        chunks = []             # chunk tiles for gathered layer-1 h
        h_last = [None, None]
        c_last = [None, None]

        for t in range(T if COMM_MODE != "merged" else 0):
            s = t % OUT_CHUNK
            ci = t // OUT_CHUNK
            if s == 0:
                chunks.append(chpool.tile([128, KT_H * OUT_CHUNK * 64], wdt,
                                          name="chunk", tag="chunk"))
            chunk = chunks[-1]

            # ---- layer 0 ----
            xt = xpool.tile([128, 512], wdt)
            nc.sync.dma_start(xt[:], xts[t])
            g0src = hg0s if t == 0 else g0_prev
            lhsT0 = [xt[:, 64 * kk:64 * (kk + 1)] for kk in range(KT_H)] + \
                    [g0src[:, 64 * kk:64 * (kk + 1)] for kk in range(KT_H)]
            c0n, h0t, hT0 = cell(lhsT0, w0s, 0, c_prev[0])
            g0 = g0pool.tile([128, 512], wdt)
            allgather(hT0, g0[:].rearrange("p (kk b) -> p kk b", kk=KT_H))

            # ---- layer 1 ----
            if t == 0:
                l1rec = [hg1s[:, 64 * kk:64 * (kk + 1)] for kk in range(KT_H)]
            else:
                pch = chunks[(t - 1) // OUT_CHUNK]
                ps_ = (t - 1) % OUT_CHUNK
                l1rec = [pch[:, 512 * kk + 64 * ps_:512 * kk + 64 * ps_ + 64]
                         for kk in range(KT_H)]
            lhsT1 = [g0[:, 64 * kk:64 * (kk + 1)] for kk in range(KT_H)] + l1rec
            c1n, h1t, hT1 = cell(lhsT1, w1s, 512, c_prev[1])
            allgather(
                hT1,
                chunk[:].rearrange("p (kk s b) -> p kk s b",
                                   kk=KT_H, s=OUT_CHUNK)[:, :, s, :])

            c_prev = [c0n, c1n]
            g0_prev = g0
            h_last = [h0t, h1t]
            c_last = [c0n, c1n]

            # ---- output projection (batched) ----
            if s == OUT_CHUNK - 1:
                py = psy.tile([128, 512], F32)
                for j in range(4):      # step-pairs
                    nc.tensor.matmul(py[:, 128 * j:128 * (j + 1)],
                                     mc(ones[0:1, 0:128]),
                                     mc(brss[0:1, 1024:1152]),
                                     start=True, stop=False)
                    for kt in range(KT_H):
                        lh = chunk[:, 512 * kt + 128 * j:512 * kt + 128 * (j + 1)]
                        nc.tensor.matmul(py[:, 128 * j:128 * (j + 1)],
                                         mc(lh), mc(wos[:, 128 * kt:128 * (kt + 1)]),
                                         start=False, stop=(kt == KT_H - 1))
                ysb = ypool.tile([128, 512], F32)
                nc.scalar.activation(ysb[:], py[:], Tanh)
                nc.sync.dma_start(yt[ci], ysb[:])

        if COMM_MODE != "merged":
            for l in range(L):
                nc.sync.dma_start(hns[l], h_last[l][:])
                nc.sync.dma_start(cns[l], c_last[l][:])

    nc.compile()
    return nc


_NC_CACHE = {}


def _get_nc(T, wdt_name):
    key = (T, wdt_name)
    if key not in _NC_CACHE:
        _NC_CACHE[key] = build_nc(T, wdt_name)
    return _NC_CACHE[key]


def _make_runner(nc):
    """Mirror of bass2jax.run_bass_via_pjrt's multi-core path, kept as a
    reusable jitted callable so executions can be repeated / timed."""
    import jax
    from jax.experimental.shard_map import shard_map
    from jax.sharding import Mesh, PartitionSpec
    from concourse import bass2jax

    bass2jax.install_neuronx_cc_hook()
    partition_name = (nc.partition_id_tensor.name
                      if nc.partition_id_tensor else None)
    in_names, out_names, out_avals = [], [], []
    for alloc in nc.m.functions[0].allocations:
        if not isinstance(alloc, mybir.MemoryLocationSet):
            continue
        name = alloc.memorylocations[0].name
        if alloc.kind == "ExternalInput":
            if name != partition_name:
                in_names.append(name)
        elif alloc.kind == "ExternalOutput":
            out_names.append(name)
            out_avals.append(jax.core.ShapedArray(
                tuple(alloc.tensor_shape), mybir.dt.np(alloc.dtype)))
    n_params = len(in_names)
    all_in = list(in_names) + list(out_names)
    if partition_name is not None:
        all_in.append(partition_name)
    donate = tuple(range(n_params, n_params + len(out_names)))

    def _body(*args):
        operands = list(args)
        if partition_name is not None:
            operands.append(bass2jax.partition_id_tensor())
        outs = bass2jax._bass_exec_p.bind(
            *operands,
            out_avals=tuple(out_avals),
            in_names=tuple(all_in),
            out_names=tuple(out_names),
            lowering_input_output_aliases=(),
            sim_require_finite=False,
            sim_require_nnan=False,
            nc=nc,
        )
        return tuple(outs)

    devices = jax.devices()[:NCORES]
    mesh = Mesh(np.asarray(devices), ("core",))
    spec = PartitionSpec("core")
    fn = jax.jit(
        shard_map(_body, mesh=mesh,
                  in_specs=(spec,) * (n_params + len(out_names)),
                  out_specs=(spec,) * len(out_names),
                  check_rep=False),
        donate_argnums=donate, keep_unused=True)
    return fn, in_names, out_names, out_avals, mesh


def run_pjrt(nc, in_maps, repeat=1):
    """Execute on the 8 cores; returns (results list, exec wall times)."""
    import jax
    from jax.sharding import NamedSharding, PartitionSpec
    import time as _time

    fn, in_names, out_names, out_avals, mesh = _make_runner(nc)
    sharding = NamedSharding(mesh, PartitionSpec("core"))
    concat_in = [
        np.concatenate([np.asarray(in_maps[c][n]) for c in range(NCORES)],
                       axis=0)
        for n in in_names
    ]
    dev_in = [jax.device_put(a, sharding) for a in concat_in]
    jax.block_until_ready(dev_in)
    times = []
    out_arrs = None
    for _ in range(max(1, repeat)):
        zeros = [
            jax.device_put(
                np.zeros((NCORES * av.shape[0], *av.shape[1:]), av.dtype),
                sharding)
            for av in out_avals
        ]
        jax.block_until_ready(zeros)
        t0 = _time.time()
        out_arrs = fn(*dev_in, *zeros)
        jax.block_until_ready(out_arrs)
        times.append(_time.time() - t0)
    results = []
    for c in range(NCORES):
        results.append({
            name: np.asarray(out_arrs[i]).reshape(
                NCORES, *out_avals[i].shape)[c]
            for i, name in enumerate(out_names)
        })
    return results, times


def run(inputs, T=T_FULL, wdt_name=None, repeat=1):
    wdt_name = wdt_name or WDT_NAME
    in_maps = pack_inputs(T=T, wdt_name=wdt_name, **inputs)
    nc = _get_nc(T, wdt_name)
    results, times = run_pjrt(nc, in_maps, repeat=repeat)
    y, hn, cn = decode_outputs(results, T)
    return (y, hn, cn), times


def kernel(x, h0, c0, Uw, Ub, Ww, Wb, Wout, bout):
    out, _ = run(dict(x=x, h0=h0, c0=c0, Uw=Uw, Ub=Ub, Ww=Ww, Wb=Wb,
                      Wout=Wout, bout=bout))
    return out


# revision 15
# speedup vs baseline: 2.1586x; 2.1586x over previous
"""Trainium2 Bass kernel for nn_CLSTM: 2-layer complex LSTM.

T=256 time steps, B=64 batch, H=512 complex hidden (1024 real dims), L=2 layers.

Strategy (8 NeuronCores, tensor-parallel over hidden channels):
  - Each core owns 64 complex channels (=128 real dims) of every gate / state.
  - Activations live batch-on-partitions: tiles are [64 batch, channels-free],
    so all complex (r,i) arithmetic is free-dim local (DVE lanes are
    partition-local on trn2 - no cross-partition ops needed).
  - Gate GEMMs: out[64, 512] (batch x 4*128 gate cols) accumulated over
    K-tiles; lhsT = transposed activations [128 chan, 64 batch] (stationary),
    rhs = weight slices [128, 512] (moving, N=512 streams at 1 col/cycle).
  - Complex linear algebra is lifted to real GEMMs with the doubled matrix
    W_big = [[wr, wi], [-wi, wr]].
  - Bias enters as a K=1 matmul of a ones-row against a bias-row.
  - h is transposed back to [128 chan, 64 batch] on the PE each step, cast,
    and AllGather'ed across the 8 cores (ncfw collective via DRAM bounce).
  - Output projection is batched 8 steps at a time (M=128 = 2 steps x 64).
  - All input/output permutation, transposition and weight packing is done
    host-side in numpy.
"""

import os
import sys
from contextlib import ExitStack

sys.path.insert(0, "/opt/trn_rl_repo")

import numpy as np
import ml_dtypes

import concourse.bass as bass
import concourse.tile as tile
import concourse.mybir as mybir
from concourse import bacc

# Problem constants
T_FULL, B, L, H = 256, 64, 2, 512
D = 2 * H            # 1024 real dims
NCORES = 8
CPC = H // NCORES    # 64 complex channels per core
RPC = 2 * CPC        # 128 real dims per core
KT_H = D // 128      # 8 K-tiles for one hidden vector
GATE_ORDER = (0, 1, 3, 2)   # col-block order: f, i, o, a (ref order 0=f,1=i,2=a,3=o)
OUT_CHUNK = 8        # outproj batching (steps per chunk)

F32 = mybir.dt.float32
BF16 = mybir.dt.bfloat16

WDT_NAME = os.environ.get("CLSTM_WDT", "f32r")   # "f32" | "f32r" | "bf16"
COMM_MODE = os.environ.get("CLSTM_COMM", "merged")  # merged | ccompute | none


def _perm():
    """sharded row order -> standard feature index."""
    p = np.zeros(D, dtype=np.int64)
    for k in range(NCORES):
        p[128 * k:128 * k + 64] = np.arange(64 * k, 64 * k + 64)
        p[128 * k + 64:128 * k + 128] = 512 + np.arange(64 * k, 64 * k + 64)
    return p


PERM = _perm()


def _wbig(w):
    """w: [2, H, H] (wr, wi) -> [2H, 2H] real matrix, std order both sides."""
    wr, wi = np.asarray(w[0]), np.asarray(w[1])
    return np.block([[wr, wi], [-wi, wr]]).astype(np.float32)


def _np_wdt(name):
    return {"f32": np.float32, "f32r": np.float32, "bf16": ml_dtypes.bfloat16}[name]


def pack_inputs(x, h0, c0, Uw, Ub, Ww, Wb, Wout, bout, T, wdt_name):
    """Build the 8 per-core input maps."""
    x = np.asarray(x, np.float32)[:T]
    h0 = np.asarray(h0, np.float32)
    c0 = np.asarray(c0, np.float32)
    Uw = np.asarray(Uw, np.float32)
    Ub = np.asarray(Ub, np.float32)
    Ww = np.asarray(Ww, np.float32)
    Wb = np.asarray(Wb, np.float32)
    Wout = np.asarray(Wout, np.float32)
    bout = np.asarray(bout, np.float32)
    nwdt = _np_wdt(wdt_name)

    # xts[t, p, kk*64+b] = x[t, b, PERM[kk*128+p]]  (shared by all cores)
    xp = x[:, :, PERM]                                    # [T, B, D]
    xts = np.ascontiguousarray(
        xp.reshape(T, B, KT_H, 128).transpose(0, 3, 2, 1).reshape(T, 128, KT_H * B)
    ).astype(nwdt)

    # hg[l, p, kk*64+b] = h0[l, b, PERM[kk*128+p]]
    h0p = h0[:, :, PERM]
    hg = np.ascontiguousarray(
        h0p.reshape(L, B, KT_H, 128).transpose(0, 3, 2, 1).reshape(L, 128, KT_H * B)
    ).astype(nwdt)

    ones = np.ones((1, 128), dtype=nwdt)
    ident = np.eye(64, dtype=np.float32)

    # Big matrices (std order), per layer/gate
    Ubig = [[_wbig(Uw[l, g]) for g in range(4)] for l in range(L)]
    Wbig = [[_wbig(Ww[l, g]) for g in range(4)] for l in range(L)]
    Obig = _wbig(Wout)

    in_maps = []
    for k in range(NCORES):
        colsel = PERM[128 * k:128 * k + 128]
        ws = []
        for l in range(L):
            cols = []
            for g in GATE_ORDER:
                top = Ubig[l][g][PERM][:, colsel]      # [1024, 128] input-x part
                bot = Wbig[l][g][PERM][:, colsel]      # [1024, 128] recurrent part
                cols.append(np.concatenate([top, bot], axis=0))   # [2048, 128]
            wcat = np.concatenate(cols, axis=1)        # [2048, 512]
            w = wcat.reshape(16, 128, 512).transpose(1, 0, 2).reshape(128, 16 * 512)
            ws.append(np.ascontiguousarray(w).astype(nwdt))

        brow = np.zeros((1, 2 * 512 + 128), dtype=np.float32)
        for l in range(L):
            bsum = Ub[l] + Wb[l]                       # [4, D]
            brow[0, 512 * l:512 * (l + 1)] = np.concatenate(
                [bsum[g][colsel] for g in GATE_ORDER])
        brow[0, 1024:1152] = bout[colsel]

        wo = Obig[PERM][:, colsel]                     # [1024, 128]
        wo = wo.reshape(8, 128, 128).transpose(1, 0, 2).reshape(128, 1024)

        in_maps.append({
            "xts": xts,
            "w0": ws[0],
            "w1": ws[1],
            "wo": np.ascontiguousarray(wo).astype(nwdt),
            "brs": brow.astype(nwdt),
            "hg": hg,
            "c0s": np.ascontiguousarray(c0[:, :, colsel]),
            "ones": ones,
            "ident": ident,
        })
    return in_maps


def decode_outputs(results, T):
    """results: list of 8 per-core out dicts -> (y, hn, cn) full arrays."""
    y = np.zeros((T, B, D), dtype=np.float32)
    hn = np.zeros((L, B, D), dtype=np.float32)
    cn = np.zeros((L, B, D), dtype=np.float32)
    nch = T // OUT_CHUNK
    for k in range(NCORES):
        colsel = PERM[128 * k:128 * k + 128]
        yt = results[k]["yt"]                          # [nch, 128, 512]
        # yt[c, s2*64+b, 128*j+m] = y[8c+2j+s2, b, colsel[m]]
        yy = yt.reshape(nch, 2, 64, 4, 128).transpose(0, 3, 1, 2, 4)
        y[:, :, colsel] = yy.reshape(T, 64, 128)
        hn[:, :, colsel] = results[k]["hns"]
        cn[:, :, colsel] = results[k]["cns"]
    return y, hn, cn


def build_nc(T, wdt_name):
    f32r_mode = wdt_name == "f32r"
    wdt = {"bf16": BF16, "f32": F32, "f32r": mybir.dt.float32r}[wdt_name]

    def mc(ap):      # matmul-dtype cast (no-op; dtypes carried on tensors)
        return ap

    def swapview(ap):
        """[P, 2h] unit-stride AP -> view reading (2nd half | 1st half)."""
        (pstep, pcnt), (fstep, fcnt) = list(ap.ap)
        assert fstep == 1 and fcnt % 2 == 0
        h = fcnt // 2
        return bass.AP(ap.tensor, ap.offset + h,
                       [[pstep, pcnt], [-h, 2], [1, h]])

    nch = T // OUT_CHUNK
    nc = bacc.Bacc("TRN2", target_bir_lowering=False, debug=False,
                   num_devices=NCORES)

    xts = nc.dram_tensor("xts", [T, 128, 512], wdt, kind="ExternalInput").ap()
    w0 = nc.dram_tensor("w0", [128, 16 * 512], wdt, kind="ExternalInput").ap()
    w1 = nc.dram_tensor("w1", [128, 16 * 512], wdt, kind="ExternalInput").ap()
    wo = nc.dram_tensor("wo", [128, 8 * 128], wdt, kind="ExternalInput").ap()
    brs = nc.dram_tensor("brs", [1, 1152], wdt, kind="ExternalInput").ap()
    hg = nc.dram_tensor("hg", [L, 128, 512], wdt, kind="ExternalInput").ap()
    c0s = nc.dram_tensor("c0s", [L, 64, 128], F32, kind="ExternalInput").ap()
    onesd = nc.dram_tensor("ones", [1, 128], wdt, kind="ExternalInput").ap()
    identd = nc.dram_tensor("ident", [64, 64], F32, kind="ExternalInput").ap()

    yt = nc.dram_tensor("yt", [nch, 128, 512], F32, kind="ExternalOutput").ap()
    hns = nc.dram_tensor("hns", [L, 64, 128], F32, kind="ExternalOutput").ap()
    cns = nc.dram_tensor("cns", [L, 64, 128], F32, kind="ExternalOutput").ap()

    rg = [list(range(NCORES))]

    with tile.TileContext(nc) as tc, ExitStack() as ctx:
        consts = ctx.enter_context(tc.tile_pool(name="consts", bufs=1))
        w0s = consts.tile([128, 16 * 512], wdt)
        nc.sync.dma_start(w0s[:], w0)
        w1s = consts.tile([128, 16 * 512], wdt)
        nc.sync.dma_start(w1s[:], w1)
        wos = consts.tile([128, 8 * 128], wdt)
        nc.sync.dma_start(wos[:], wo)
        brss = consts.tile([1, 1152], wdt)
        nc.sync.dma_start(brss[:], brs)
        ones = consts.tile([1, 128], wdt)
        nc.sync.dma_start(ones[:], onesd)
        ident = consts.tile([64, 64], F32)
        nc.sync.dma_start(ident[:], identd)
        hg0s = consts.tile([128, 512], wdt)
        nc.sync.dma_start(hg0s[:], hg[0])
        hg1s = consts.tile([128, 512], wdt)
        nc.sync.dma_start(hg1s[:], hg[1])

        xpool = ctx.enter_context(tc.tile_pool(name="xst", bufs=6))
        psc = ctx.enter_context(tc.tile_pool(name="psc", bufs=4, space="PSUM"))
        pst_p = ctx.enter_context(tc.tile_pool(name="pst", bufs=2, space="PSUM"))
        psy = ctx.enter_context(tc.tile_pool(name="psy", bufs=1, space="PSUM"))
        gpool = ctx.enter_context(tc.tile_pool(name="gates", bufs=3))
        tmp = ctx.enter_context(tc.tile_pool(name="tmp", bufs=3))
        cpool = ctx.enter_context(tc.tile_pool(name="cstate", bufs=3))
        hpool = ctx.enter_context(tc.tile_pool(name="hstate", bufs=3))
        htp = ctx.enter_context(tc.tile_pool(name="ht", bufs=3))
        g0pool = ctx.enter_context(tc.tile_pool(name="g0", bufs=3))
        chpool = ctx.enter_context(tc.tile_pool(name="chunk", bufs=2))
        ypool = ctx.enter_context(tc.tile_pool(name="ysb", bufs=2))
        dbin = ctx.enter_context(tc.tile_pool(name="dbin", bufs=4, space="DRAM"))
        dbout = ctx.enter_context(tc.tile_pool(name="dbout", bufs=4, space="DRAM"))

        # initial c states
        c_prev = []
        for l in range(L):
            ct = cpool.tile([64, 128], F32)
            nc.sync.dma_start(ct[:], c0s[l])
            c_prev.append(ct)

        Sig = mybir.ActivationFunctionType.Sigmoid
        Tanh = mybir.ActivationFunctionType.Tanh

        def cell(lhsT_tiles, wsb, boff, c_old):
            """One complex-LSTM cell for this core's channel slice.

            lhsT_tiles: list of 16 APs [128, 64] (transposed inputs, wdt)
            wsb: weight SBUF tile [128, 16*512]; boff: bias col offset
            c_old: [64, 128] f32
            returns (c_new, h [64,128] f32, hT [128,64] wdt)
            """
            ps = psc.tile([64, 512], F32)
            nc.tensor.matmul(ps[:], mc(ones[0:1, 0:64]), mc(brss[0:1, boff:boff + 512]),
                             start=True, stop=False)
            for kt in range(16):
                nc.tensor.matmul(ps[:], mc(lhsT_tiles[kt]),
                                 mc(wsb[:, 512 * kt:512 * (kt + 1)]),
                                 start=False, stop=(kt == 15))
            gt = gpool.tile([64, 512], F32)
            nc.scalar.activation(gt[:, 0:384], ps[:, 0:384], Sig)
            nc.scalar.activation(gt[:, 384:512], ps[:, 384:512], Tanh)
            f = gt[:, 0:128]
            ig = gt[:, 128:256]
            o = gt[:, 256:384]
            a = gt[:, 384:512]

            c, v = c_old, nc.vector
            A = tmp.tile([64, 128], F32)
            v.tensor_mul(A[:], c[:], f)                       # (crfr | cifi)
            Q = tmp.tile([64, 128], F32)
            v.tensor_mul(Q[:], a, ig)                         # (arir | aiii)
            V1 = tmp.tile([64, 128], F32)
            v.tensor_add(V1[:], A[:], Q[:])
            CX = tmp.tile([64, 128], F32)
            v.tensor_mul(CX[:], swapview(c[:]), f)            # (cifr | crfi)
            DX = tmp.tile([64, 128], F32)
            v.tensor_mul(DX[:], swapview(a), ig)              # (aiir | arii)
            V2 = tmp.tile([64, 128], F32)
            v.tensor_add(V2[:], CX[:], DX[:])
            c_new = cpool.tile([64, 128], F32)
            v.tensor_sub(c_new[:, 0:64], V1[:, 0:64], V1[:, 64:128])
            v.tensor_add(c_new[:, 64:128], V2[:, 0:64], V2[:, 64:128])

            th = tmp.tile([64, 128], F32)
            nc.scalar.activation(th[:], c_new[:], Tanh)
            E = tmp.tile([64, 128], F32)
            v.tensor_mul(E[:], o, th[:])                      # (or*thr | oi*thi)
            FX = tmp.tile([64, 128], F32)
            v.tensor_mul(FX[:], swapview(o), th[:])           # (oithr | orthi)
            h = hpool.tile([64, 128], F32)
            v.tensor_sub(h[:, 0:64], E[:, 0:64], E[:, 64:128])
            v.tensor_add(h[:, 64:128], FX[:, 0:64], FX[:, 64:128])

            pt = pst_p.tile([128, 64], F32)
            nc.tensor.transpose(pt[:], h[:], ident[:])
            hT = htp.tile([128, 64], wdt)
            nc.scalar.copy(hT[:], pt[:])
            return c_new, h, hT

        def allgather(hT, dest_ap):
            """hT [128,64] sbuf -> gathered into dest_ap ([128, 8, 64] view)."""
            if COMM_MODE == "none":
                # timing-only stub: local broadcast (WRONG results)
                for kk in range(NCORES):
                    nc.sync.dma_start(dest_ap[:, kk, :], hT[:])
                return
            bi = dbin.tile([128, 64], wdt)
            nc.sync.dma_start(bi[:], hT[:])
            bo = dbout.tile([NCORES * 128, 64], wdt)
            nc.gpsimd.collective_compute(
                "AllGather", mybir.AluOpType.bypass, replica_groups=rg,
                ins=[bi[:].opt()], outs=[bo[:].opt()])
            nc.sync.dma_start(
                dest_ap, bo[:].rearrange("(kk p) b -> p kk b", kk=NCORES))

        if COMM_MODE == "merged":
            # Skewed schedule: one AllGather per iteration carrying
            # (h_l0(i), h_l1(i-1)).
            g0_ring = {}        # i -> gathered h_l0(i) [128, 512] tile
            chunks = []
            h_last = [None, None]
            c_last = [None, None]
            hT0_i = hT1_prev = None
            for i in range(T + 1):
                if i >= 1 and (i - 1) % 8 == 0:
                    chunks.append(chpool.tile([128, KT_H * OUT_CHUNK * 64],
                                              wdt, name="chunk", tag="chunk"))
                if i < T:
                    xt = xpool.tile([128, 512], wdt)
                    nc.sync.dma_start(xt[:], xts[i])
                    g0src = hg0s if i == 0 else g0_ring[i - 1]
                    lhsT0 = [xt[:, 64 * kk:64 * (kk + 1)] for kk in range(KT_H)] + \
                            [g0src[:, 64 * kk:64 * (kk + 1)] for kk in range(KT_H)]
                    c0n, h0t, hT0_i = cell(lhsT0, w0s, 0, c_prev[0])
                    c_prev[0] = c0n
                    h_last[0], c_last[0] = h0t, c0n
                if i >= 1:
                    g0 = g0_ring[i - 1]
                    if i == 1:
                        l1rec = [hg1s[:, 64 * kk:64 * (kk + 1)]
                                 for kk in range(KT_H)]
                    else:
                        pch = chunks[(i - 2) // OUT_CHUNK]
                        ps_ = (i - 2) % OUT_CHUNK
                        l1rec = [pch[:, 512 * kk + 64 * ps_:512 * kk + 64 * ps_ + 64]
                                 for kk in range(KT_H)]
                    lhsT1 = [g0[:, 64 * kk:64 * (kk + 1)]
                             for kk in range(KT_H)] + l1rec
                    c1n, h1t, hT1_prev = cell(lhsT1, w1s, 512, c_prev[1])
                    c_prev[1] = c1n
                    h_last[1], c_last[1] = h1t, c1n

                # merged AllGather: slot 0 = h_l0(i), slot 1 = h_l1(i-1)
                bi = dbin.tile([256, 64], wdt, name="bi", tag="bi")
                nc.sync.dma_start(bi[0:128, :],
                                  (hT0_i if i < T else hT1_prev)[:])
                nc.sync.dma_start(bi[128:256, :],
                                  (hT1_prev if i >= 1 else hT0_i)[:])
                bo = dbout.tile([NCORES * 256, 64], wdt, name="bo", tag="bo")
                nc.gpsimd.collective_compute(
                    "AllGather", mybir.AluOpType.bypass, replica_groups=rg,
                    ins=[bi[:].opt()], outs=[bo[:].opt()])
                bview = bo[:].rearrange("(kk s p) b -> p s kk b",
                                        kk=NCORES, s=2)
                if i < T:
                    g0 = g0pool.tile([128, 512], wdt)
                    nc.sync.dma_start(
                        g0[:].rearrange("p (kk b) -> p kk b", kk=KT_H),
                        bview[:, 0])
                    g0_ring[i] = g0
                    g0_ring.pop(i - 3, None)
                if i >= 1:
                    j = i - 1
                    ch = chunks[j // OUT_CHUNK]
                    nc.sync.dma_start(
                        ch[:].rearrange("p (kk s b) -> p kk s b",
                                        kk=KT_H, s=OUT_CHUNK)[:, :, j % 8, :],
                        bview[:, 1])
                # outproj for chunk c once its last slot (step 8c+7) landed
                if i >= 8 and i % 8 == 0:
                    ci = (i - 8) // OUT_CHUNK
                    chunk = chunks[ci]
                    py = psy.tile([128, 512], F32)
                    for j in range(4):
                        nc.tensor.matmul(py[:, 128 * j:128 * (j + 1)],
                                         mc(ones[0:1, 0:128]),
                                         mc(brss[0:1, 1024:1152]),
                                         start=True, stop=False)
                        for kt in range(KT_H):
                            lh = chunk[:, 512 * kt + 128 * j:
                                       512 * kt + 128 * (j + 1)]
                            nc.tensor.matmul(py[:, 128 * j:128 * (j + 1)],
                                             mc(lh),
                                             mc(wos[:, 128 * kt:128 * (kt + 1)]),
                                             start=False, stop=(kt == KT_H - 1))
                    ysb = ypool.tile([128, 512], F32)
                    nc.scalar.activation(ysb[:], py[:], Tanh)
                    nc.sync.dma_start(yt[ci], ysb[:])

            for l in range(L):
                nc.sync.dma_start(hns[l], h_last[l][:])
                nc.sync.dma_start(cns[l], c_last[l][:])

        g0_prev = None          # gathered h for layer 0, [128, 512] tile
        chunks = []             # chunk tiles for gathered layer-1 h
        h_last = [None, None]
        c_last = [None, None]

        for t in range(T if COMM_MODE != "merged" else 0):
            s = t % OUT_CHUNK
            ci = t // OUT_CHUNK
            if s == 0:
                chunks.append(chpool.tile([128, KT_H * OUT_CHUNK * 64], wdt,
                                          name="chunk", tag="chunk"))
            chunk = chunks[-1]

            # ---- layer 0 ----
            xt = xpool.tile([128, 512], wdt)
            nc.sync.dma_start(xt[:], xts[t])
            g0src = hg0s if t == 0 else g0_prev
            lhsT0 = [xt[:, 64 * kk:64 * (kk + 1)] for kk in range(KT_H)] + \
                    [g0src[:, 64 * kk:64 * (kk + 1)] for kk in range(KT_H)]
            c0n, h0t, hT0 = cell(lhsT0, w0s, 0, c_prev[0])
            g0 = g0pool.tile([128, 512], wdt)
            allgather(hT0, g0[:].rearrange("p (kk b) -> p kk b", kk=KT_H))

            # ---- layer 1 ----
            if t == 0:
                l1rec = [hg1s[:, 64 * kk:64 * (kk + 1)] for kk in range(KT_H)]
            else:
                pch = chunks[(t - 1) // OUT_CHUNK]
                ps_ = (t - 1) % OUT_CHUNK
                l1rec = [pch[:, 512 * kk + 64 * ps_:512 * kk + 64 * ps_ + 64]
                         for kk in range(KT_H)]
            lhsT1 = [g0[:, 64 * kk:64 * (kk + 1)] for kk in range(KT_H)] + l1rec
            c1n, h1t, hT1 = cell(lhsT1, w1s, 512, c_prev[1])
            allgather(
                hT1,
                chunk[:].rearrange("p (kk s b) -> p kk s b",
                                   kk=KT_H, s=OUT_CHUNK)[:, :, s, :])

            c_prev = [c0n, c1n]
            g0_prev = g0
            h_last = [h0t, h1t]
            c_last = [c0n, c1n]

            # ---- output projection (batched) ----
            if s == OUT_CHUNK - 1:
                py = psy.tile([128, 512], F32)
                for j in range(4):      # step-pairs
                    nc.tensor.matmul(py[:, 128 * j:128 * (j + 1)],
                                     mc(ones[0:1, 0:128]),
                                     mc(brss[0:1, 1024:1152]),
                                     start=True, stop=False)
                    for kt in range(KT_H):
                        lh = chunk[:, 512 * kt + 128 * j:512 * kt + 128 * (j + 1)]
                        nc.tensor.matmul(py[:, 128 * j:128 * (j + 1)],
                                         mc(lh), mc(wos[:, 128 * kt:128 * (kt + 1)]),
                                         start=False, stop=(kt == KT_H - 1))
                ysb = ypool.tile([128, 512], F32)
                nc.scalar.activation(ysb[:], py[:], Tanh)
                nc.sync.dma_start(yt[ci], ysb[:])

        if COMM_MODE != "merged":
            for l in range(L):
                nc.sync.dma_start(hns[l], h_last[l][:])
                nc.sync.dma_start(cns[l], c_last[l][:])

    nc.compile()
    return nc


_NC_CACHE = {}


def _get_nc(T, wdt_name):
    key = (T, wdt_name)
    if key not in _NC_CACHE:
        _NC_CACHE[key] = build_nc(T, wdt_name)
    return _NC_CACHE[key]


def _make_runner(nc):
    """Mirror of bass2jax.run_bass_via_pjrt's multi-core path, kept as a
    reusable jitted callable so executions can be repeated / timed."""
    import jax
    from jax.experimental.shard_map import shard_map
    from jax.sharding import Mesh, PartitionSpec
    from concourse import bass2jax

    bass2jax.install_neuronx_cc_hook()
    partition_name = (nc.partition_id_tensor.name
                      if nc.partition_id_tensor else None)
    in_names, out_names, out_avals = [], [], []
    for alloc in nc.m.functions[0].allocations:
        if not isinstance(alloc, mybir.MemoryLocationSet):
            continue
        name = alloc.memorylocations[0].name
        if alloc.kind == "ExternalInput":
            if name != partition_name:
                in_names.append(name)
        elif alloc.kind == "ExternalOutput":
            out_names.append(name)
            out_avals.append(jax.core.ShapedArray(
                tuple(alloc.tensor_shape), mybir.dt.np(alloc.dtype)))
    n_params = len(in_names)
    all_in = list(in_names) + list(out_names)
    if partition_name is not None:
        all_in.append(partition_name)
    donate = tuple(range(n_params, n_params + len(out_names)))

    def _body(*args):
        operands = list(args)
        if partition_name is not None:
            operands.append(bass2jax.partition_id_tensor())
        outs = bass2jax._bass_exec_p.bind(
            *operands,
            out_avals=tuple(out_avals),
            in_names=tuple(all_in),
            out_names=tuple(out_names),
            lowering_input_output_aliases=(),
            sim_require_finite=False,
            sim_require_nnan=False,
            nc=nc,
        )
        return tuple(outs)

    devices = jax.devices()[:NCORES]
    mesh = Mesh(np.asarray(devices), ("core",))
    spec = PartitionSpec("core")
    fn = jax.jit(
        shard_map(_body, mesh=mesh,
                  in_specs=(spec,) * (n_params + len(out_names)),
                  out_specs=(spec,) * len(out_names),
                  check_rep=False),
        donate_argnums=donate, keep_unused=True)
    return fn, in_names, out_names, out_avals, mesh


def run_pjrt(nc, in_maps, repeat=1):
    """Execute on the 8 cores; returns (results list, exec wall times)."""
    import jax
    from jax.sharding import NamedSharding, PartitionSpec
    import time as _time

    fn, in_names, out_names, out_avals, mesh = _make_runner(nc)
    sharding = NamedSharding(mesh, PartitionSpec("core"))
    concat_in = [
        np.concatenate([np.asarray(in_maps[c][n]) for c in range(NCORES)],
                       axis=0)
        for n in in_names
    ]
    dev_in = [jax.device_put(a, sharding) for a in concat_in]
    jax.block_until_ready(dev_in)
    times = []
    out_arrs = None
    for _ in range(max(1, repeat)):
        zeros = [
            jax.device_put(
                np.zeros((NCORES * av.shape[0], *av.shape[1:]), av.dtype),
                sharding)
            for av in out_avals
        ]
        jax.block_until_ready(zeros)
        t0 = _time.time()
        out_arrs = fn(*dev_in, *zeros)
        jax.block_until_ready(out_arrs)
        times.append(_time.time() - t0)
    results = []
    for c in range(NCORES):
        results.append({
            name: np.asarray(out_arrs[i]).reshape(
                NCORES, *out_avals[i].shape)[c]
            for i, name in enumerate(out_names)
        })
    return results, times


def run(inputs, T=T_FULL, wdt_name=None, repeat=1):
    wdt_name = wdt_name or WDT_NAME
    in_maps = pack_inputs(T=T, wdt_name=wdt_name, **inputs)
    nc = _get_nc(T, wdt_name)
    results, times = run_pjrt(nc, in_maps, repeat=repeat)
    y, hn, cn = decode_outputs(results, T)
    return (y, hn, cn), times


def kernel(x, h0, c0, Uw, Ub, Ww, Wb, Wout, bout):
    out, _ = run(dict(x=x, h0=h0, c0=c0, Uw=Uw, Ub=Ub, Ww=Ww, Wb=Wb,
                      Wout=Wout, bout=bout))
    return out
